# revision 5
# baseline (speedup 1.0000x reference)
"""EntropyBottleneck forward (q_mode='noise') as a Trainium2 Bass kernel.

Math
----
reference computes, per channel c with tiny per-channel params (W_k, b_k, f_k):

    y    = x + noise
    v    = y flattened per channel
    L(v) = chain of FactorizeCell: u <- softplus(W_k) @ u + b_k,
           then u <- u + tanh(f_k) * tanh(u)   (for k < last)
    lower = L(v - 0.5); upper = L(v + 0.5)
    s     = -sign(lower + upper)
    lik   = max(|sigmoid(s*upper) - sigmoid(s*lower)|, 1e-9)

When every gate f_k == 0 (true for this module's initialization), the chain is
per-channel *affine*: L(v) = M_c * v + D_c, with M_c > 0. Then with t = M*y+D,
h = M/2 and the tanh identity (sign trick folds away):

    lik = 0.5 * (tanh((t + h)/2) - tanh((t - h)/2))

h/2 = M/4 is tiny (M = 0.1 for this init), so the central difference collapses
to the derivative with relative error <= (h/2)^2 * max|g'''/g'| / 6 ~ 2e-4
(g = tanh), far inside the 2e-2 gate:

    lik = (M/4) * sech^2(t/2) = (M/4) * (1 - tanh^2(t/2))

The 1e-9 lower bound is a provable no-op: |t| <= |D| + M*max|y| < 2 keeps
lik >= 0.01.  Device work per element:

    y = x + n                       (vector, fp16 2x)
    w = tanh((M/2)*y + D/2)         (scalar engine, per-partition scale/bias)
    lik = (-M/4)*w^2 + (M/4)        (vector: square fp16 2x, then
                                     tensor_scalar with per-partition APs)

I/O is fp16 (the 2e-2 tolerance gives >=10x margin; fp16 keeps y's absolute
error ~8e-3 even at |y|~10): per-core HBM traffic drops from 12.6 MB to
6.3 MB.  x|noise are packed into one [384, 4096] fp16 input (single
dma_start per 128-row tile, 8 KB descriptors), y|lik into one [384, 4096]
fp16 output, stored per half-tile as compute completes.

Sharding: data-parallel over batch, one batch element per NeuronCore.
Per-core tensor (192, 4096) is viewed as (384, 2048): row r holds half of
channel r//2, so per-channel coefficients are per-partition scale/bias APs.
"""

import numpy as np

B, C, H, W = 8, 192, 64, 64
NCORES = 8
ROWS, COLS = 384, 2048  # (C, H*W) = (192, 4096) viewed as (384, 2048)
NT = ROWS // 128  # 3 row-tiles of 128 partitions
PACK = 2 * COLS  # packed row: [x | noise] in, [y h0 | lik h0 | y h1 | lik h1] out
HALF = COLS // 2

_CACHE: dict = {}


def _softplus64(x: np.ndarray) -> np.ndarray:
    x = x.astype(np.float64)
    return np.log1p(np.exp(-np.abs(x))) + np.maximum(x, 0.0)


def _fold_affine(ws, bs):
    """Compose the per-channel affine chain: L(v) = M*v + D. Returns (M, D) as (C,)."""
    M = np.ones((C, 1, 1), np.float64)
    D = np.zeros((C, 1, 1), np.float64)
    for Wk, bk in zip(ws, bs):
        spw = _softplus64(np.asarray(Wk))
        M = spw @ M
        D = spw @ D + np.asarray(bk, np.float64)
    return M[:, 0, 0], D[:, 0, 0]


def _numpy_fallback(x, noise, ws, bs, fs):
    """Exact replica of the reference chain for the general (gated) case."""
    x = np.asarray(x, np.float32)
    noise = np.asarray(noise, np.float32)
    y = x + noise
    v = y.transpose(1, 0, 2, 3).reshape(C, 1, -1).astype(np.float32)

    def logits(v):
        for i, (Wk, bk) in enumerate(zip(ws, bs)):
            spw = _softplus64(np.asarray(Wk)).astype(np.float32)
            v = np.einsum("coi,cin->con", spw, v) + np.asarray(bk, np.float32)
            if i < len(fs):
                v = v + np.tanh(np.asarray(fs[i], np.float32)) * np.tanh(v)
        return v

    lower = logits(v - 0.5)
    upper = logits(v + 0.5)
    sign = -np.sign(lower + upper)
    sig = lambda z: 1.0 / (1.0 + np.exp(-z, dtype=np.float32))
    lik = np.abs(sig(sign * upper) - sig(sign * lower))
    lik = np.maximum(lik, np.float32(1e-9))
    lik = lik.reshape(C, B, H, W).transpose(1, 0, 2, 3)
    return y, lik


def _build_fp16():
    """Hand-scheduled fp16 kernel: explicit per-engine streams + semaphores.

    sync   : xn tile loads, then per-half-tile packed y|lik stores (SP FIFO)
    scalar : param load (ACT FIFO), then one tanh per half-tile chunk
    vector : y = x+n adds, then w^2 and the (-M/4)w^2 + M/4 affine per chunk
    """
    import concourse.bacc as bacc
    import concourse.mybir as mybir

    f32, f16 = mybir.dt.float32, mybir.dt.float16
    nc = bacc.Bacc("TRN2", target_bir_lowering=False, debug=False,
                   num_devices=NCORES)

    xn_d = nc.dram_tensor("xn", [ROWS, PACK], f16, kind="ExternalInput")
    # prm cols: [0:NT] = D/2, [NT:2NT] = M/2, [2NT:3NT] = -M/4, [3NT:4NT] = M/4
    prm_d = nc.dram_tensor("prm", [128, 4 * NT], f32, kind="ExternalInput")
    o_d = nc.dram_tensor("out", [ROWS, PACK], f16, kind="ExternalOutput")

    Tanh = mybir.ActivationFunctionType.Tanh
    op_add = mybir.AluOpType.add
    op_mult = mybir.AluOpType.mult

    NCH = 2          # compute chunks per tile (half-tiles of 1024 cols)
    NG = NT * NCH    # 6 chunks; chunk i = (tile i//2, half i%2)

    xns = [nc.alloc_sbuf_tensor(f"xn{t}", [128, PACK], f16) for t in range(NT)]
    outs = [nc.alloc_sbuf_tensor(f"o{t}", [128, PACK], f16) for t in range(NT)]
    wts = [nc.alloc_sbuf_tensor(f"w{i}", [128, HALF], f16) for i in range(NG)]
    prm = nc.alloc_sbuf_tensor("prms", [128, 4 * NT], f32)

    # Packed 2048-col slabs, one per chunk i = (t, h) at cols [h*2048, (h+1)*2048):
    #   in  slab: [x_h (1024) | n_h (1024)]     out slab: [y_h (1024) | lik_h (1024)]
    # so every load, store and compute chunk is one contiguous [128, <=2048] AP.
    def xcols(h):
        return slice(h * COLS, h * COLS + HALF)

    def ncols(h):
        return slice(h * COLS + HALF, (h + 1) * COLS)

    ycols = xcols
    lcols = ncols

    def scols(h):  # full in/out slab for chunk h of a tile
        return slice(h * COLS, (h + 1) * COLS)

    # One semaphore per load chunk, +16 per transfer (one per SDMA engine).
    ld = [nc.alloc_semaphore(f"ld{i}") for i in range(NG)]
    ldp = nc.alloc_semaphore("ldp")  # params
    va = nc.alloc_semaphore("va")    # vector adds (+1 each, engine-ordered)
    sa = nc.alloc_semaphore("sa")    # scalar tanhs (+1 each, engine-ordered)
    vt = nc.alloc_semaphore("vt")    # square+affine chains (+1 per chunk)
    st = nc.alloc_semaphore("st")    # store completions

    def ld_chunk(q, i):
        t, h = divmod(i, NCH)
        rows = slice(t * 128, (t + 1) * 128)
        q.dma_start(xns[t][:, scols(h)],
                    xn_d[rows, scols(h)]).then_inc(ld[i], 16)

    # No SWDGE (gpsimd) DMAs are issued, so skip GpSimd's expensive
    # dge_drain at block exit (~3.5-4us).
    with nc.Block(no_gpsimd_drain=True) as block:

        @block.sync
        def _(sync):
            # Loads lead on both HWDGE FIFOs (chunks 1,3 issue from the ACT
            # FIFO) so all descriptors enqueue ahead of every store's and the
            # first chunk lands ~2.5us earlier than a whole-tile load would.
            for i in (0, 2, 4, 5):
                ld_chunk(sync, i)
            # Stores in chunk order; each waits only on its own compute.
            for i in range(NG):
                t, h = divmod(i, NCH)
                rows = slice(t * 128, (t + 1) * 128)
                sync.wait_ge(vt, i + 1)
                sync.dma_start(o_d[rows, scols(h)],
                               outs[t][:, scols(h)]).then_inc(st, 16)
            sync.wait_ge(st, NG * 16)

        @block.vector
        def _(vector):
            def add(i):
                t, h = divmod(i, NCH)
                vector.wait_ge(ld[i], 16)
                nc.vector.tensor_tensor(outs[t][:, ycols(h)], xns[t][:, xcols(h)],
                                        xns[t][:, ncols(h)],
                                        op=op_add).then_inc(va, 1)

            def lik(i):
                t, h = divmod(i, NCH)
                vector.wait_ge(sa, i + 1)
                nc.vector.tensor_tensor(wts[i][:], wts[i][:], wts[i][:],
                                        op=op_mult)
                nc.vector.tensor_scalar(outs[t][:, lcols(h)], wts[i][:],
                                        prm[:, 2 * NT + t:2 * NT + t + 1],
                                        prm[:, 3 * NT + t:3 * NT + t + 1],
                                        op0=op_mult, op1=op_add).then_inc(vt, 1)

            add(0)
            add(1)
            lik(0)
            add(2)
            lik(1)
            add(3)
            lik(2)
            add(4)
            lik(3)
            add(5)
            lik(4)
            lik(5)

        @block.scalar
        def _(scalar):
            scalar.dma_start(prm[:], prm_d[:]).then_inc(ldp, 16)
            ld_chunk(scalar, 1)
            ld_chunk(scalar, 3)
            scalar.wait_ge(ldp, 16)
            for i in range(NG):
                t, h = divmod(i, NCH)
                scalar.wait_ge(va, i + 1)
                nc.scalar.activation(wts[i][:], outs[t][:, ycols(h)], Tanh,
                                     bias=prm[:, t:t + 1],
                                     scale=prm[:, NT + t:NT + t + 1]).then_inc(sa, 1)

    nc.compile()
    return nc


def _get_program():
    if "nc" not in _CACHE:
        _CACHE["nc"] = _build_fp16()
    return _CACHE["nc"]


def _prep_in_maps(x, noise, M, D):
    """Pack fp16 inputs + per-partition params into per-core in_maps."""
    ch = np.arange(ROWS) // 2  # channel id per folded row
    Mr, Dr = M[ch], D[ch]
    prm = np.empty((128, 4 * NT), np.float32)
    prm[:, 0 * NT:1 * NT] = (Dr / 2).astype(np.float32).reshape(NT, 128).T
    prm[:, 1 * NT:2 * NT] = (Mr / 2).astype(np.float32).reshape(NT, 128).T
    prm[:, 2 * NT:3 * NT] = (-Mr / 4).astype(np.float32).reshape(NT, 128).T
    prm[:, 3 * NT:4 * NT] = (Mr / 4).astype(np.float32).reshape(NT, 128).T

    x16 = np.asarray(x, np.float16).reshape(NCORES, ROWS, 2, HALF)
    n16 = np.asarray(noise, np.float16).reshape(NCORES, ROWS, 2, HALF)
    # slab layout per row: [x h0 | n h0 | x h1 | n h1] in 1024-col blocks
    xn = np.empty((NCORES, ROWS, 2, 2, HALF), np.float16)
    xn[:, :, :, 0, :] = x16
    xn[:, :, :, 1, :] = n16
    xn = xn.reshape(NCORES, ROWS, PACK)
    return [{"xn": xn[b], "prm": prm} for b in range(NCORES)]


def _unpack_outputs(res):
    """Split packed per-core [384, 4096] fp16 outputs into full y, lik fp32."""
    o = np.stack([res[b]["out"] for b in range(NCORES)])  # (8, 384, 4096) fp16
    o = o.reshape(NCORES, ROWS, 2, 2, HALF)
    y = o[:, :, :, 0, :].reshape(NCORES, C, H, W).astype(np.float32)
    lik = o[:, :, :, 1, :].reshape(NCORES, C, H, W).astype(np.float32)
    return y, lik


def kernel(x, noise, w0, b0, f0, w1, b1, f1, w2, b2, f2, w3, b3):
    from concourse.bass_utils import run_bass_kernel_spmd

    ws = [w0, w1, w2, w3]
    bs = [b0, b1, b2, b3]
    fs = [f0, f1, f2]

    if any(np.any(np.asarray(f) != 0.0) for f in fs):
        # Gated (non-affine) case: bit-accurate host fallback. Never taken for
        # this module's initialization (all gates are zero).
        return _numpy_fallback(x, noise, ws, bs, fs)

    M, D = _fold_affine(ws, bs)  # (C,) float64 each, M > 0
    if np.ptp(M) > 1e-9 * np.abs(M).max() or np.abs(M).max() > 1.0:
        # The central-difference step (M/4) is validated for small M only;
        # fall back for out-of-family parameters (never hit for this module).
        return _numpy_fallback(x, noise, ws, bs, fs)

    x = np.ascontiguousarray(np.asarray(x, np.float32))
    noise = np.ascontiguousarray(np.asarray(noise, np.float32))

    nc = _get_program()
    in_maps = _prep_in_maps(x, noise, M, D)
    res = run_bass_kernel_spmd(nc, in_maps, list(range(NCORES))).results
    return _unpack_outputs(res)


# revision 7
# speedup vs baseline: 1.0682x; 1.0682x over previous
"""EntropyBottleneck forward (q_mode='noise') as a Trainium2 Bass kernel.

Math
----
reference computes, per channel c with tiny per-channel params (W_k, b_k, f_k):

    y    = x + noise
    v    = y flattened per channel
    L(v) = chain of FactorizeCell: u <- softplus(W_k) @ u + b_k,
           then u <- u + tanh(f_k) * tanh(u)   (for k < last)
    lower = L(v - 0.5); upper = L(v + 0.5)
    s     = -sign(lower + upper)
    lik   = max(|sigmoid(s*upper) - sigmoid(s*lower)|, 1e-9)

When every gate f_k == 0 (true for this module's initialization), the chain is
per-channel *affine*: L(v) = M_c * v + D_c, with M_c > 0. Then with t = M*y+D,
h = M/2 and the tanh identity (sign trick folds away):

    lik = 0.5 * (tanh((t + h)/2) - tanh((t - h)/2))

h/2 = M/4 is tiny (M = 0.1 for this init), so the central difference collapses
to the derivative with relative error <= (h/2)^2 * max|g'''/g'| / 6 ~ 2e-4
(g = tanh), far inside the 2e-2 gate:

    lik = (M/4) * sech^2(t/2) = (M/4) * (1 - tanh^2(t/2))

The 1e-9 lower bound is a provable no-op: |t| <= |D| + M*max|y| < 2 keeps
lik >= 0.01.  Device work per element:

    y = x + n                       (vector, fp16 2x)
    w = tanh((M/2)*y + D/2)         (scalar engine, per-partition scale/bias)
    lik = (-M/4)*w^2 + (M/4)        (vector: square fp16 2x, then
                                     tensor_scalar with per-partition APs)

I/O is fp16 (the 2e-2 tolerance gives >=10x margin; fp16 keeps y's absolute
error ~8e-3 even at |y|~10): per-core HBM traffic drops from 12.6 MB to
6.3 MB.  x|noise are packed into one [384, 4096] fp16 input (single
dma_start per 128-row tile, 8 KB descriptors), y|lik into one [384, 4096]
fp16 output, stored per half-tile as compute completes.

Sharding: data-parallel over batch, one batch element per NeuronCore.
Per-core tensor (192, 4096) is viewed as (384, 2048): row r holds half of
channel r//2, so per-channel coefficients are per-partition scale/bias APs.
"""

import numpy as np

B, C, H, W = 8, 192, 64, 64
NCORES = 8
ROWS, COLS = 384, 2048  # (C, H*W) = (192, 4096) viewed as (384, 2048)
NT = ROWS // 128  # 3 row-tiles of 128 partitions
PACK = 2 * COLS  # packed row: [x | noise] in, [y h0 | lik h0 | y h1 | lik h1] out
HALF = COLS // 2

_CACHE: dict = {}


def _softplus64(x: np.ndarray) -> np.ndarray:
    x = x.astype(np.float64)
    return np.log1p(np.exp(-np.abs(x))) + np.maximum(x, 0.0)


def _fold_affine(ws, bs):
    """Compose the per-channel affine chain: L(v) = M*v + D. Returns (M, D) as (C,)."""
    M = np.ones((C, 1, 1), np.float64)
    D = np.zeros((C, 1, 1), np.float64)
    for Wk, bk in zip(ws, bs):
        spw = _softplus64(np.asarray(Wk))
        M = spw @ M
        D = spw @ D + np.asarray(bk, np.float64)
    return M[:, 0, 0], D[:, 0, 0]


def _numpy_fallback(x, noise, ws, bs, fs):
    """Exact replica of the reference chain for the general (gated) case."""
    x = np.asarray(x, np.float32)
    noise = np.asarray(noise, np.float32)
    y = x + noise
    v = y.transpose(1, 0, 2, 3).reshape(C, 1, -1).astype(np.float32)

    def logits(v):
        for i, (Wk, bk) in enumerate(zip(ws, bs)):
            spw = _softplus64(np.asarray(Wk)).astype(np.float32)
            v = np.einsum("coi,cin->con", spw, v) + np.asarray(bk, np.float32)
            if i < len(fs):
                v = v + np.tanh(np.asarray(fs[i], np.float32)) * np.tanh(v)
        return v

    lower = logits(v - 0.5)
    upper = logits(v + 0.5)
    sign = -np.sign(lower + upper)
    sig = lambda z: 1.0 / (1.0 + np.exp(-z, dtype=np.float32))
    lik = np.abs(sig(sign * upper) - sig(sign * lower))
    lik = np.maximum(lik, np.float32(1e-9))
    lik = lik.reshape(C, B, H, W).transpose(1, 0, 2, 3)
    return y, lik


def _build_fp16():
    """Hand-scheduled fp16 kernel: explicit per-engine streams + semaphores.

    sync   : xn tile loads, then per-half-tile packed y|lik stores (SP FIFO)
    scalar : param load (ACT FIFO), then one tanh per half-tile chunk
    vector : y = x+n adds, then w^2 and the (-M/4)w^2 + M/4 affine per chunk
    """
    import concourse.bacc as bacc
    import concourse.mybir as mybir

    f32, f16 = mybir.dt.float32, mybir.dt.float16
    nc = bacc.Bacc("TRN2", target_bir_lowering=False, debug=False,
                   num_devices=NCORES)

    xn_d = nc.dram_tensor("xn", [ROWS, PACK], f16, kind="ExternalInput")
    # prm cols: [0:NT] = D/2, [NT:2NT] = M/2, [2NT:3NT] = -M/4, [3NT:4NT] = M/4
    prm_d = nc.dram_tensor("prm", [128, 4 * NT], f32, kind="ExternalInput")
    o_d = nc.dram_tensor("out", [ROWS, PACK], f16, kind="ExternalOutput")

    Tanh = mybir.ActivationFunctionType.Tanh
    op_add = mybir.AluOpType.add
    op_mult = mybir.AluOpType.mult

    NCH = 2          # compute chunks per tile (half-tiles of 1024 cols)
    NG = NT * NCH    # 6 chunks; chunk i = (tile i//2, half i%2)

    xns = [nc.alloc_sbuf_tensor(f"xn{t}", [128, PACK], f16) for t in range(NT)]
    outs = [nc.alloc_sbuf_tensor(f"o{t}", [128, PACK], f16) for t in range(NT)]
    wts = [nc.alloc_sbuf_tensor(f"w{i}", [128, HALF], f16) for i in range(NG)]
    prm = nc.alloc_sbuf_tensor("prms", [128, 4 * NT], f32)

    # Packed 2048-col slabs, one per chunk i = (t, h) at cols [h*2048, (h+1)*2048):
    #   in  slab: [x_h (1024) | n_h (1024)]     out slab: [y_h (1024) | lik_h (1024)]
    # so every load, store and compute chunk is one contiguous [128, <=2048] AP.
    def xcols(h):
        return slice(h * COLS, h * COLS + HALF)

    def ncols(h):
        return slice(h * COLS + HALF, (h + 1) * COLS)

    ycols = xcols
    lcols = ncols

    def scols(h):  # full in/out slab for chunk h of a tile
        return slice(h * COLS, (h + 1) * COLS)

    # One semaphore per load chunk, +16 per transfer (one per SDMA engine).
    ld = [nc.alloc_semaphore(f"ld{i}") for i in range(NG)]
    ldp = nc.alloc_semaphore("ldp")  # params
    va = nc.alloc_semaphore("va")    # vector adds (+1 each, engine-ordered)
    sa = nc.alloc_semaphore("sa")    # scalar tanhs (+1 each, engine-ordered)
    vt = nc.alloc_semaphore("vt")    # square+affine chains (+1 per chunk)
    st = nc.alloc_semaphore("st")    # store completions

    def ld_chunk(q, i):
        t, h = divmod(i, NCH)
        rows = slice(t * 128, (t + 1) * 128)
        q.dma_start(xns[t][:, scols(h)],
                    xn_d[rows, scols(h)]).then_inc(ld[i], 16)

    # No SWDGE (gpsimd) DMAs are issued, so skip GpSimd's expensive
    # dge_drain at block exit (~3.5-4us).
    with nc.Block(no_gpsimd_drain=True) as block:

        @block.sync
        def _(sync):
            # All bulk loads on the SP FIFO, in chunk order: descriptors
            # enqueue ahead of every store's, and the first chunk lands
            # ~2.5us earlier than a whole-tile load would. (Bulk loads on
            # the ACT FIFO interleave badly: engines alternate between the
            # two DGE queues, roughly doubling each chunk's completion.)
            for i in range(NG):
                ld_chunk(sync, i)
            # Stores in chunk order; each waits only on its own compute.
            for i in range(NG):
                t, h = divmod(i, NCH)
                rows = slice(t * 128, (t + 1) * 128)
                sync.wait_ge(vt, i + 1)
                sync.dma_start(o_d[rows, scols(h)],
                               outs[t][:, scols(h)]).then_inc(st, 16)
            sync.wait_ge(st, NG * 16)

        @block.vector
        def _(vector):
            def add(i):
                t, h = divmod(i, NCH)
                vector.wait_ge(ld[i], 16)
                nc.vector.tensor_tensor(outs[t][:, ycols(h)], xns[t][:, xcols(h)],
                                        xns[t][:, ncols(h)],
                                        op=op_add).then_inc(va, 1)

            def lik(i):
                t, h = divmod(i, NCH)
                vector.wait_ge(sa, i + 1)
                nc.vector.tensor_tensor(wts[i][:], wts[i][:], wts[i][:],
                                        op=op_mult)
                nc.vector.tensor_scalar(outs[t][:, lcols(h)], wts[i][:],
                                        prm[:, 2 * NT + t:2 * NT + t + 1],
                                        prm[:, 3 * NT + t:3 * NT + t + 1],
                                        op0=op_mult, op1=op_add).then_inc(vt, 1)

            add(0)
            add(1)
            lik(0)
            add(2)
            lik(1)
            add(3)
            lik(2)
            add(4)
            lik(3)
            add(5)
            lik(4)
            lik(5)

        @block.scalar
        def _(scalar):
            scalar.dma_start(prm[:], prm_d[:]).then_inc(ldp, 16)
            scalar.wait_ge(ldp, 16)
            for i in range(NG):
                t, h = divmod(i, NCH)
                scalar.wait_ge(va, i + 1)
                nc.scalar.activation(wts[i][:], outs[t][:, ycols(h)], Tanh,
                                     bias=prm[:, t:t + 1],
                                     scale=prm[:, NT + t:NT + t + 1]).then_inc(sa, 1)

    nc.compile()
    return nc


def _get_program():
    if "nc" not in _CACHE:
        _CACHE["nc"] = _build_fp16()
    return _CACHE["nc"]


def _prep_in_maps(x, noise, M, D):
    """Pack fp16 inputs + per-partition params into per-core in_maps."""
    ch = np.arange(ROWS) // 2  # channel id per folded row
    Mr, Dr = M[ch], D[ch]
    prm = np.empty((128, 4 * NT), np.float32)
    prm[:, 0 * NT:1 * NT] = (Dr / 2).astype(np.float32).reshape(NT, 128).T
    prm[:, 1 * NT:2 * NT] = (Mr / 2).astype(np.float32).reshape(NT, 128).T
    prm[:, 2 * NT:3 * NT] = (-Mr / 4).astype(np.float32).reshape(NT, 128).T
    prm[:, 3 * NT:4 * NT] = (Mr / 4).astype(np.float32).reshape(NT, 128).T

    x16 = np.asarray(x, np.float16).reshape(NCORES, ROWS, 2, HALF)
    n16 = np.asarray(noise, np.float16).reshape(NCORES, ROWS, 2, HALF)
    # slab layout per row: [x h0 | n h0 | x h1 | n h1] in 1024-col blocks
    xn = np.empty((NCORES, ROWS, 2, 2, HALF), np.float16)
    xn[:, :, :, 0, :] = x16
    xn[:, :, :, 1, :] = n16
    xn = xn.reshape(NCORES, ROWS, PACK)
    return [{"xn": xn[b], "prm": prm} for b in range(NCORES)]


def _unpack_outputs(res):
    """Split packed per-core [384, 4096] fp16 outputs into full y, lik fp32."""
    o = np.stack([res[b]["out"] for b in range(NCORES)])  # (8, 384, 4096) fp16
    o = o.reshape(NCORES, ROWS, 2, 2, HALF)
    y = o[:, :, :, 0, :].reshape(NCORES, C, H, W).astype(np.float32)
    lik = o[:, :, :, 1, :].reshape(NCORES, C, H, W).astype(np.float32)
    return y, lik


def kernel(x, noise, w0, b0, f0, w1, b1, f1, w2, b2, f2, w3, b3):
    from concourse.bass_utils import run_bass_kernel_spmd

    ws = [w0, w1, w2, w3]
    bs = [b0, b1, b2, b3]
    fs = [f0, f1, f2]

    if any(np.any(np.asarray(f) != 0.0) for f in fs):
        # Gated (non-affine) case: bit-accurate host fallback. Never taken for
        # this module's initialization (all gates are zero).
        return _numpy_fallback(x, noise, ws, bs, fs)

    M, D = _fold_affine(ws, bs)  # (C,) float64 each, M > 0
    if np.ptp(M) > 1e-9 * np.abs(M).max() or np.abs(M).max() > 1.0:
        # The central-difference step (M/4) is validated for small M only;
        # fall back for out-of-family parameters (never hit for this module).
        return _numpy_fallback(x, noise, ws, bs, fs)

    x = np.ascontiguousarray(np.asarray(x, np.float32))
    noise = np.ascontiguousarray(np.asarray(noise, np.float32))

    nc = _get_program()
    in_maps = _prep_in_maps(x, noise, M, D)
    res = run_bass_kernel_spmd(nc, in_maps, list(range(NCORES))).results
    return _unpack_outputs(res)
